# revision 20
# baseline (speedup 1.0000x reference)
"""Trainium2 Bass kernel for nn_DecoderLayer (self-attn + cross-attn + FFN).

Sharding: 8 cores, no collectives. Core c handles batch b=c//2, query-row
half r=c%2 (512 of 1024 rows). All per-core differences flow through input
data (host slices/transposes/permutes), so one SPMD NEFF serves all cores.

On-device layout is feature-major ("transposed"): activations live as
[channels(partitions), tokens(free)]. Weights are host-pre-transposed to
[in_ch, out_ch] and cast to bf16. Matmul operands are bf16 (fp32 PSUM
accumulation); the residual stream stays fp32.

Structure vs the straightforward version:
- Softmax denominators are FUSED into the attn@V matmul: V is stored
  per-head as 65 columns [V | 16.0], so the accumulated pav row 64 is
  16*sum(exp); its reciprocal normalizes both attn-out and the head-mean
  prob accumulation (out-proj weights are host-prescaled 16x). This
  removes all ones-matmul denominator passes from the PE.
- Scores go into double-wide [128,2,512] PSUM tiles so one Activation
  instruction exponentiates 1024 columns (halves Act overhead).
- LN stats (sum / sum-sq over channel=partition) are ones-matmuls into
  rows 0/32 of one PSUM bank; rsqrt is exp(-0.5*ln(var+eps)) so only the
  natural_log_exp activation table is needed until the gelu switch.
- Head-mean probs (self_w/cross_w) accumulate in one bf16 partial via a
  pair tree split across DVE and Pool (gpsimd); converted to f32 at dump.
- Cross-attn K projections fill self-attention's softmax-chain PE gaps;
  cross V projections run from out-proj onward (so the padded-V buffer
  can share one SBUF slot) and fill the LN2 gap, Q2, and early cross
  pair boundaries.
- Weight streams alternate two SBUF rings (wpA/wpB) so a phase's DMAs
  never wait on the previous phase's readers; w2 is host-relayouted to
  [8, 128, 4096] so FFN2 needs one contiguous 2KB-line DMA per chunk.
"""

from collections import deque

import ml_dtypes
import numpy as np

import concourse.bacc as bacc
import concourse.mybir as mybir
import concourse.tile as tile
from concourse.bass_utils import run_bass_kernel_spmd

F32 = mybir.dt.float32
BF16 = mybir.dt.bfloat16
AF = mybir.ActivationFunctionType
OP = mybir.AluOpType

P = 128
D = 1024
DFF = 4096
H = 16
B = 4
L = 1024          # full sequence (keys/values)
LQ = 512          # per-core query tokens
NC = D // P       # 8 channel chunks
NF = DFF // P     # 32 ff chunks
NSC = L // P      # 8 key-position chunks
EPS = 1e-5


def _build():
    nc = bacc.Bacc("TRN2", target_bir_lowering=False)

    xT16 = nc.dram_tensor("xT16", [D, L], BF16, kind="ExternalInput")
    xaT16 = nc.dram_tensor("xaT16", [D, L], BF16, kind="ExternalInput")
    w_sa = nc.dram_tensor("w_sa", [D, 3 * D], BF16, kind="ExternalInput")
    b_sa = nc.dram_tensor("b_sa", [3 * D], F32, kind="ExternalInput")
    wo_sa = nc.dram_tensor("wo_sa", [D, D], BF16, kind="ExternalInput")
    bo_sa = nc.dram_tensor("bo_sa", [D], F32, kind="ExternalInput")
    w_ca = nc.dram_tensor("w_ca", [D, 3 * D], BF16, kind="ExternalInput")
    b_ca = nc.dram_tensor("b_ca", [3 * D], F32, kind="ExternalInput")
    wo_ca = nc.dram_tensor("wo_ca", [D, D], BF16, kind="ExternalInput")
    bo_ca = nc.dram_tensor("bo_ca", [D], F32, kind="ExternalInput")
    w1 = nc.dram_tensor("w1", [D, DFF], BF16, kind="ExternalInput")
    b1 = nc.dram_tensor("b1", [DFF], F32, kind="ExternalInput")
    w2t = nc.dram_tensor("w2t", [NC, P, DFF], BF16, kind="ExternalInput")
    b2 = nc.dram_tensor("b2", [D], F32, kind="ExternalInput")
    ln_w = nc.dram_tensor("ln_w", [3, D], F32, kind="ExternalInput")
    ln_b = nc.dram_tensor("ln_b", [3, D], F32, kind="ExternalInput")

    xoutT = nc.dram_tensor("xoutT", [D, LQ], F32, kind="ExternalOutput")
    selfwT = nc.dram_tensor("selfwT", [L, LQ], F32, kind="ExternalOutput")
    crosswT = nc.dram_tensor("crosswT", [L, LQ], F32, kind="ExternalOutput")

    with tile.TileContext(nc) as tc:
        _emit(nc, tc, locals())
    nc.compile()
    return nc


def _emit(nc, tc, t):
    import contextlib
    ctx = contextlib.ExitStack()
    with ctx:
        const = ctx.enter_context(tc.tile_pool(name="const", bufs=1))
        big = ctx.enter_context(tc.tile_pool(name="big", bufs=1))
        wpA = ctx.enter_context(tc.tile_pool(name="wpA", bufs=8))
        wpB = ctx.enter_context(tc.tile_pool(name="wpB", bufs=8))
        w2p = ctx.enter_context(tc.tile_pool(name="w2p", bufs=2))
        sm = ctx.enter_context(tc.tile_pool(name="sm", bufs=1))      # [1,512] rows
        rep = ctx.enter_context(tc.tile_pool(name="rep", bufs=1))    # broadcast tiles
        expp = ctx.enter_context(tc.tile_pool(name="expp", bufs=8))  # [P,2,512] exps
        outp = ctx.enter_context(tc.tile_pool(name="outp", bufs=2))  # transients
        psu = ctx.enter_context(tc.tile_pool(name="psu", bufs=1, space="PSUM"))

        # ---- constants ----
        lnw_sb = const.tile([P, 3, NC], F32, name="lnw_sb")
        nc.sync.dma_start(out=lnw_sb, in_=t["ln_w"].rearrange("k (o p) -> p k o", p=P))
        lnb_sb = const.tile([P, 3, NC], F32, name="lnb_sb")
        nc.sync.dma_start(out=lnb_sb, in_=t["ln_b"].rearrange("k (o p) -> p k o", p=P))
        bqk_sa = const.tile([P, 16], F32, name="bqk_sa")
        nc.sync.dma_start(out=bqk_sa, in_=t["b_sa"][: 2 * D].rearrange("(o p) -> p o", p=P))
        bqk_ca = const.tile([P, 16], F32, name="bqk_ca")
        nc.sync.dma_start(out=bqk_ca, in_=t["b_ca"][: 2 * D].rearrange("(o p) -> p o", p=P))
        bo_sa_sb = const.tile([P, NC], F32, name="bo_sa_sb")
        nc.sync.dma_start(out=bo_sa_sb, in_=t["bo_sa"].rearrange("(o p) -> p o", p=P))
        bo_ca_sb = const.tile([P, NC], F32, name="bo_ca_sb")
        nc.sync.dma_start(out=bo_ca_sb, in_=t["bo_ca"].rearrange("(o p) -> p o", p=P))
        b1_sb = const.tile([P, NF], F32, name="b1_sb")
        nc.sync.dma_start(out=b1_sb, in_=t["b1"].rearrange("(o p) -> p o", p=P))
        b2_sb = const.tile([P, NC], F32, name="b2_sb")
        nc.sync.dma_start(out=b2_sb, in_=t["b2"].rearrange("(o p) -> p o", p=P))
        eps_sb = const.tile([P, 1], F32, name="eps_sb")
        nc.vector.memset(eps_sb, EPS)
        ones_sb = const.tile([P, 1], BF16, name="ones_sb")
        nc.vector.memset(ones_sb, 1.0)
        # v-bias rows replicated across partitions, [P, H, 64]
        bv_reps = {}
        for key in ("sa", "ca"):
            bv_rep = const.tile([P, H, 64], BF16, name=f"bv_{key}_rep")
            for j in range(2):
                row = sm.tile([1, 512], F32, name=f"bv_{key}_row", tag="row")
                nc.sync.dma_start(
                    out=row, in_=t[f"b_{key}"][None, 2 * D + 512 * j: 2 * D + 512 * j + 512])
                row16 = sm.tile([1, 512], BF16, name=f"bv_{key}_row16", tag="row16", bufs=2)
                nc.vector.tensor_copy(row16, row)
                nc.gpsimd.partition_broadcast(
                    bv_rep[:, 8 * j: 8 * j + 8, :],
                    row16.rearrange("q (h e) -> q h e", e=64))
            bv_reps[key] = bv_rep

        def stream_w(pool, dram, k, lo, hi, name):
            w_t = pool.tile([P, hi - lo], BF16, name=name, tag="wp")
            nc.sync.dma_start(out=w_t, in_=dram[P * k: P * k + P, lo:hi])
            return w_t

        # ================= LayerNorm helpers =================
        def ln_stats(x_chunk, st, o, is_bf16):
            """Accumulate sum (row 0) / sumsq (row 32) of one [P,512] chunk."""
            if is_bf16:
                xb = x_chunk
            else:
                xb = outp.tile([P, 512], BF16, name="ln_xb", tag="lnt")
                nc.vector.tensor_copy(xb, x_chunk)
            sq = outp.tile([P, 512], BF16, name="ln_sq", tag="lnt")
            nc.scalar.activation(sq, x_chunk, AF.Square)
            nc.tensor.matmul(st[0:1, :], ones_sb, xb,
                             start=(o == 0), stop=(o == NC - 1),
                             skip_group_check=True)
            nc.tensor.matmul(st[32:33, :], ones_sb, sq,
                             start=(o == 0), stop=(o == NC - 1),
                             skip_group_check=True)

        def ln_finish(x_sl, st, ln_idx, out_b, out_f, name):
            """Normalize NC chunks of 512 tokens using stats in st rows 0/32.

            The psum stat rows are partition-broadcast (gpsimd crosses
            partitions legally); the whole prologue then runs full-width,
            which costs the same (free-dim bound) and needs no further
            broadcasts."""
            rows = outp.tile([P, 512], F32, name=name + "_rows", tag="lnu",
                             bufs=2)
            nc.vector.tensor_scalar_mul(rows[0:1, :], st[0:1, :], 1.0 / D)
            nc.vector.tensor_copy(rows[32:33, :], st[32:33, :])
            qrow = sm.tile([1, 512], F32, name=name + "_qrow", tag="row")
            nc.sync.dma_start(out=qrow, in_=rows[32:33, :])
            mean_rep = rep.tile([P, 512], F32, name=name + "_mrep", tag="rep",
                                bufs=2)
            nc.gpsimd.partition_broadcast(mean_rep, rows[0:1, :])
            rsq_rep = rep.tile([P, 512], F32, name=name + "_rrep", tag="rep",
                               bufs=2)
            nc.gpsimd.partition_broadcast(rsq_rep, qrow)
            m2 = outp.tile([P, 512], F32, name=name + "_m2", tag="lnu", bufs=2)
            nc.vector.tensor_tensor(m2, mean_rep, mean_rep, OP.mult)
            nc.vector.scalar_tensor_tensor(rsq_rep, rsq_rep, 1.0 / D, m2,
                                           OP.mult, OP.subtract)
            nc.scalar.activation(m2, rsq_rep, AF.Ln, bias=eps_sb)
            nc.scalar.activation(rsq_rep, m2, AF.Exp, scale=-0.5)
            for o in range(NC):
                u = outp.tile([P, 512], F32, name=name + "_u", tag="lnu", bufs=2)
                nc.vector.tensor_tensor(u, x_sl(o), mean_rep, OP.subtract)
                v = u
                nc.vector.scalar_tensor_tensor(
                    v, u, lnw_sb[:, ln_idx, o: o + 1], rsq_rep, OP.mult, OP.mult)
                if out_f is not None:
                    nc.scalar.activation(out_f(o), v, AF.Identity,
                                         bias=lnb_sb[:, ln_idx, o: o + 1])
                    nc.vector.tensor_copy(out_b(o), out_f(o))
                else:
                    nc.scalar.activation(out_b(o), v, AF.Identity,
                                         bias=lnb_sb[:, ln_idx, o: o + 1])

        # ================= projection helpers =================
        def q_proj(xq_sl, wch, bqk, qT, tagpfx):
            for m in range(NC):
                acc = psu.tile([P, 512], F32, name=tagpfx + "qps", tag="pa", bufs=3)
                for k in range(NC):
                    nc.tensor.matmul(acc, wch[k][:, 128 * m: 128 * m + 128],
                                     xq_sl(k), start=(k == 0), stop=(k == NC - 1),
                                     skip_group_check=True)
                nc.scalar.activation(qT[:, m, :], acc, AF.Identity,
                                     bias=bqk[:, m: m + 1])

        def k_proj_iter(wch, xkv_b, bqk, kT, m, j):
            acc = psu.tile([P, 512], F32, name="kps", tag="pa", bufs=3)
            for k in range(NC):
                nc.tensor.matmul(
                    acc, wch[k][:, 128 * m: 128 * m + 128],
                    xkv_b[:, k, 512 * j: 512 * j + 512],
                    start=(k == 0), stop=(k == NC - 1), skip_group_check=True)
            nc.scalar.activation(kT[:, m, 512 * j: 512 * j + 512], acc,
                                 AF.Identity, bias=bqk[:, 8 + m: 9 + m])

        def v_proj_iter(wch, xkv_b, bv_rep, vnat, m, j):
            """vnat: [P, NSC, H, 65]; writes heads 8j..8j+8 of token-chunk m."""
            acc = psu.tile([P, 512], F32, name="vps", tag="pa", bufs=3)
            for k in range(NC):
                nc.tensor.matmul(
                    acc, xkv_b[:, k, 128 * m: 128 * m + 128],
                    wch[k][:, 512 * j: 512 * j + 512],
                    start=(k == 0), stop=(k == NC - 1), skip_group_check=True)
            nc.vector.tensor_tensor(
                vnat[:, m, 8 * j: 8 * j + 8, 0:64],
                acc.rearrange("p (h e) -> p h e", e=64),
                bv_rep[:, 8 * j: 8 * j + 8, :], OP.add)

        # ================= attention =================
        def attention(qT, kT, vnat, part, tagpfx, fillers, drain_by):
            """aoT [P, NC, LQ] bf16; head-mean probs accumulate into bf16
            part [P, NSC, LQ]. Fillers (independent PE work) are drained by
            pair boundary `drain_by`."""
            aoT = big.tile([P, NC, LQ], BF16, name=tagpfx + "aoT", tag="aoTr")
            deferred = []
            npairs = H // 2
            for g in range(npairs):
                recs = []
                pair_e2 = []
                for hh in range(2):
                    h = 2 * g + hh
                    base = 64 * hh
                    pav = psu.tile([65, 512], F32, name=tagpfx + "pav",
                                   tag="pa", bufs=3)
                    e2s = []
                    for scp in range(4):
                        pss = psu.tile([P, 2, 512], F32, name=tagpfx + "pss",
                                       tag="sc", bufs=2)
                        for half in range(2):
                            sc = 2 * scp + half
                            nc.tensor.matmul(
                                pss[:, half, :],
                                kT[base: base + 64, g, 128 * sc: 128 * sc + 128],
                                qT[base: base + 64, g, :],
                                start=True, stop=True, skip_group_check=True)
                        e2 = expp.tile([P, 2, 512], BF16, name=tagpfx + "exp",
                                       tag="exp", bufs=8)
                        nc.scalar.activation(e2, pss, AF.Exp, scale=0.125)
                        e2s.append(e2)
                        if scp >= 1:
                            ep = e2s[scp - 1]
                            for half in range(2):
                                sc = 2 * (scp - 1) + half
                                nc.tensor.matmul(
                                    pav, vnat[:, sc, h, :], ep[:, half, :],
                                    start=(sc == 0), stop=False,
                                    skip_group_check=True)
                    for half in range(2):
                        sc = 6 + half
                        nc.tensor.matmul(
                            pav, vnat[:, sc, h, :], e2s[3][:, half, :],
                            start=False, stop=(sc == 7), skip_group_check=True)
                    rec_sb = outp.tile([P, 512], BF16, name=tagpfx + "recsb",
                                       tag="lnt", bufs=2)
                    with nc.allow_low_precision(reason="prob-scale is bf16 anyway"):
                        nc.vector.reciprocal(rec_sb[64:65, :], pav[64:65, :])
                    rec_row = sm.tile([1, 512], BF16, name=tagpfx + "rrow",
                                      tag="row16", bufs=2)
                    nc.sync.dma_start(out=rec_row, in_=rec_sb[64:65, :])
                    rec_rep = rep.tile([P, 512], BF16, name=tagpfx + "r16rep",
                                       tag="rep16", bufs=2)
                    nc.gpsimd.partition_broadcast(rec_rep, rec_row)
                    recs.append(rec_rep)
                    pair_e2.append(e2s)
                    if hh == 0:
                        nc.vector.tensor_tensor(
                            aoT[0:64, g, :], pav[0:64, :],
                            rec_rep[0:64, :], OP.mult)
                    else:
                        # partition shift 0-63 -> 64-127 must go through DMA
                        sh = outp.tile([64, 512], BF16, name=tagpfx + "aosh",
                                       tag="aosh", bufs=1)
                        nc.vector.tensor_tensor(sh, pav[0:64, :],
                                                rec_rep[0:64, :], OP.mult)
                        nc.sync.dma_start(out=aoT[64:128, g, :], in_=sh)

                def swacc_work(g=g, pair_e2=pair_e2, recs=recs):
                    # scale exps by 1/(16*sum) in place (av already consumed
                    # them), then accumulate the pair into the bf16 partial
                    for scp in range(4):
                        eng0 = nc.gpsimd if scp == 1 else nc.vector
                        scl0 = pair_e2[0][scp]
                        eng0.tensor_tensor(scl0, scl0,
                                           recs[0][:, None, :].broadcast_to([P, 2, 512]),
                                           OP.mult)
                        scl1 = pair_e2[1][scp]
                        nc.vector.tensor_tensor(scl1, scl1,
                                                recs[1][:, None, :].broadcast_to([P, 2, 512]),
                                                OP.mult)
                        dst = part[:, 2 * scp: 2 * scp + 2, :]
                        if g == 0:
                            nc.vector.tensor_tensor(dst, scl0, scl1, OP.add)
                        else:
                            pr = outp.tile([P, 2, 512], BF16, name=tagpfx + "pr",
                                           tag="pr", bufs=1)
                            nc.vector.tensor_tensor(pr, scl0, scl1, OP.add)
                            (nc.gpsimd if scp == 3 else nc.vector).tensor_tensor(
                                dst, dst, pr, OP.add)

                if g < npairs - 1:
                    swacc_work()
                else:
                    deferred.append(swacc_work)
                if fillers:
                    if g < drain_by:
                        take = -(-len(fillers) // max(1, drain_by - g))
                    else:
                        take = len(fillers)
                    for _ in range(min(take, len(fillers))):
                        fillers.popleft()()
            return aoT, deferred

        def dump_part(part, dram):
            for sc in range(NSC):
                w = outp.tile([P, 512], F32, name="dumpw", tag="lnu", bufs=2)
                nc.scalar.activation(w, part[:, sc, :], AF.Identity)
                nc.sync.dma_start(
                    out=dram.rearrange("(o p) n -> p o n", p=P)[:, sc, :],
                    in_=w)

        def out_proj(aoT, wch, bo, resid_f32, xnew, tagpfx, fillers, stats_cb):
            for m in range(NC):
                if m in (1, 3, 5, 7) and fillers:
                    fillers.popleft()()
                acc = psu.tile([P, 512], F32, name=tagpfx + "ops", tag="pa", bufs=3)
                for k in range(NC):
                    nc.tensor.matmul(acc, wch[k][:, 128 * m: 128 * m + 128],
                                     aoT[:, k, :], start=(k == 0), stop=(k == NC - 1),
                                     skip_group_check=True)
                nc.vector.scalar_tensor_tensor(
                    xnew[:, m, :], acc, bo[:, m: m + 1], resid_f32[:, m, :],
                    OP.add, OP.add)
                if stats_cb is not None and m >= 1:
                    stats_cb(xnew[:, m - 1, :], m - 1)
            if stats_cb is not None:
                stats_cb(xnew[:, NC - 1, :], NC - 1)

        # ================= pipeline =================
        # x queries-half DMA, then early weight issue, then LN1 j0
        xh0 = big.tile([P, NC, 512], BF16, name="xh0", tag="xh")
        for o in range(NC):
            nc.sync.dma_start(out=xh0[:, o, :],
                              in_=t["xT16"][P * o: P * o + P, 0:512])
        wq_sa = [stream_w(wpA, t["w_sa"], k, 0, D, "wqsa") for k in range(NC)]

        xln_b = big.tile([P, NC, L], BF16, name="xln_b", tag="lnA")
        xlnq_f = big.tile([P, NC, LQ], F32, name="xlnq_f", tag="resQ")
        st0 = psu.tile([33, 512], F32, name="ln1_st0", tag="den", bufs=1)
        for o in range(NC):
            ln_stats(xh0[:, o, :], st0, o, is_bf16=True)
        ln_finish(lambda o: xh0[:, o, :], st0, 0,
                  lambda o: xln_b[:, o, 0:512],
                  lambda o: xlnq_f[:, o, :], "ln1j0")

        # keys-half DMA + more weights; q-proj PE work overlaps the transfer
        xh1 = big.tile([P, NC, 512], BF16, name="xh1", tag="xh")
        for o in range(NC):
            nc.sync.dma_start(out=xh1[:, o, :],
                              in_=t["xT16"][P * o: P * o + P, 512:1024])
        wk_sa = [stream_w(wpB, t["w_sa"], k, D, 2 * D, "wksa") for k in range(NC)]
        xa_b = big.tile([P, NC, L], BF16, name="xa_b", tag="xaB")
        for o in range(NC):
            nc.sync.dma_start(out=xa_b[:, o, :],
                              in_=t["xaT16"][P * o: P * o + P, :])

        st1 = psu.tile([33, 512], F32, name="ln1_st1", tag="den", bufs=1)
        for o in range(NC):
            ln_stats(xh1[:, o, :], st1, o, is_bf16=True)

        qT = big.tile([P, NC, LQ], BF16, name="qT", tag="qTr")
        q_proj(lambda k: xln_b[:, k, 0:LQ], wq_sa, bqk_sa, qT, "sa")

        ln_finish(lambda o: xh1[:, o, :], st1, 0,
                  lambda o: xln_b[:, o, 512:1024], None, "ln1j1")

        wv_sa = [stream_w(wpA, t["w_sa"], k, 2 * D, 3 * D, "wvsa") for k in range(NC)]

        kT = big.tile([P, NC, L], BF16, name="sakT", tag="kTsa")
        for m in range(NC):
            for j in range(2):
                k_proj_iter(wk_sa, xln_b, bqk_sa, kT, m, j)
        wk_ca = [stream_w(wpB, t["w_ca"], k, D, 2 * D, "wkca") for k in range(NC)]

        vnat = big.tile([P, NSC, H, 65], BF16, name="vnat", tag="vnat")
        for _m in range(NSC):
            nc.gpsimd.memset(vnat[:, _m, :, 64:65], 16.0)
        for m in range(NSC):
            for j in range(2):
                v_proj_iter(wv_sa, xln_b, bv_reps["sa"], vnat, m, j)
        wo_sa_t = [stream_w(wpA, t["wo_sa"], k, 0, D, "wosa") for k in range(NC)]

        # cross-K fillers bridge self-attn softmax stalls
        k2T = big.tile([P, NC, L], BF16, name="cakT", tag="bigA")

        def k2_iter(m, j):
            def f():
                k_proj_iter(wk_ca, xa_b, bqk_ca, k2T, m, j)
            return f

        sa_fillers = deque(k2_iter(m, j) for m in range(NC) for j in range(2))

        partA = big.tile([P, NSC, LQ], BF16, name="partA", tag="xh")
        aoT, sa_deferred = attention(qT, kT, vnat, partA, "sa", sa_fillers, 8)

        # cross V: shares the vnat slot, so all writes happen after
        # self-attn's last av matmul (out-proj onward)
        wv_ca = [stream_w(wpB, t["w_ca"], k, 2 * D, 3 * D, "wvca") for k in range(NC)]
        v2nat = big.tile([P, NSC, H, 65], BF16, name="v2nat", tag="vnat")

        def v2_iter(m, j):
            def f():
                v_proj_iter(wv_ca, xa_b, bv_reps["ca"], v2nat, m, j)
            return f

        v2_fillers = deque(v2_iter(m, j) for j in range(2) for m in range(NSC))

        st2 = psu.tile([33, 512], F32, name="ln2_st", tag="den", bufs=1)

        def ln2_stats(x_chunk, o):
            ln_stats(x_chunk, st2, o, is_bf16=True)

        x1 = big.tile([P, NC, LQ], BF16, name="x1", tag="resX1")
        out_proj(aoT, wo_sa_t, bo_sa_sb, xlnq_f, x1, "sa", deque(), ln2_stats)
        for _m in range(NSC):
            nc.gpsimd.memset(v2nat[:, _m, :, 64:65], 16.0)
        for _ in range(4):
            v2_fillers.popleft()()

        x2ln_b = big.tile([P, NC, LQ], BF16, name="x2ln_b", tag="lnA")
        ln_finish(lambda o: x1[:, o, :], st2, 1,
                  lambda o: x2ln_b[:, o, :], None, "ln2")

        wq_ca = [stream_w(wpA, t["w_ca"], k, 0, D, "wqca") for k in range(NC)]

        q2T = big.tile([P, NC, LQ], BF16, name="q2T", tag="qTr")
        q_proj(lambda k: x2ln_b[:, k, :], wq_ca, bqk_ca, q2T, "ca")
        for _ in range(4):
            v2_fillers.popleft()()
        # last self pair's prob-mean work lands here, overlapping Q2-proj PE
        for work in sa_deferred:
            work()
        dump_part(partA, t["selfwT"])
        wo_ca_t = [stream_w(wpA, t["wo_ca"], k, 0, D, "woca") for k in range(NC)]

        partA2 = big.tile([P, NSC, LQ], BF16, name="partA2", tag="xh")
        ao2T, ca_deferred = attention(q2T, k2T, v2nat, partA2, "ca",
                                      v2_fillers, 4)

        w1g = [[stream_w(wpB, t["w1"], k, 0, 1024, "w1g0") for k in range(NC)]]

        st3 = psu.tile([33, 512], F32, name="ln3_st", tag="den", bufs=1)

        def ln3_stats(x_chunk, o):
            ln_stats(x_chunk, st3, o, is_bf16=False)

        x2 = big.tile([P, NC, LQ], F32, name="x2", tag="resQ")
        out_proj(ao2T, wo_ca_t, bo_ca_sb, x1, x2, "ca", deque(), ln3_stats)
        w1g.append([stream_w(wpA, t["w1"], k, 1024, 2048, "w1g1")
                    for k in range(NC)])
        x3ln_b = big.tile([P, NC, LQ], BF16, name="x3ln_b", tag="lnA")
        ln_finish(lambda o: x2[:, o, :], st3, 2,
                  lambda o: x3ln_b[:, o, :], None, "ln3")
        # last cross pair's prob-mean work overlaps FFN-f1 PE
        for work in ca_deferred:
            work()
        dump_part(partA2, t["crosswT"])

        # ================= FFN =================
        h1a = big.tile([P, NF // 2, LQ], BF16, name="h1a", tag="bigA")
        h1b = big.tile([P, NF // 2, LQ], BF16, name="h1b", tag="xaB")
        for mg in range(4):
            if mg == 2:
                w1g.append([stream_w(wpB, t["w1"], k, 2048, 3072, "w1g2")
                            for k in range(NC)])
            elif mg == 3:
                w1g.append([stream_w(wpA, t["w1"], k, 3072, 4096, "w1g3")
                            for k in range(NC)])
            wch = w1g[mg]
            h1 = h1a if mg < 2 else h1b
            for ml in range(8):
                m = 8 * mg + ml
                acc = psu.tile([P, 512], F32, name="f1ps", tag="pa", bufs=3)
                for k in range(NC):
                    nc.tensor.matmul(acc, wch[k][:, 128 * ml: 128 * ml + 128],
                                     x3ln_b[:, k, :], start=(k == 0),
                                     stop=(k == NC - 1), skip_group_check=True)
                nc.scalar.activation(h1[:, m % 16, :], acc, AF.Gelu,
                                     bias=b1_sb[:, m: m + 1])
        # FFN down + residual; w2t gives one contiguous DMA per m
        for m in range(NC):
            blks = []
            for hb in range(2):
                blk = w2p.tile([P, NF // 2, 128], BF16, name="w2blk", tag="w2")
                nc.sync.dma_start(
                    out=blk,
                    in_=t["w2t"][m][:, 2048 * hb: 2048 * hb + 2048].rearrange(
                        "p (kk n) -> p kk n", n=128))
                blks.append(blk)
            acc = psu.tile([P, 512], F32, name="f2ps", tag="pa", bufs=3)
            for k in range(NF):
                h1 = h1a if k < 16 else h1b
                nc.tensor.matmul(acc, blks[k // 16][:, k % 16, :], h1[:, k % 16, :],
                                 start=(k == 0), stop=(k == NF - 1),
                                 skip_group_check=True)
            xo = outp.tile([P, 512], F32, name="xo", tag="lnu", bufs=2)
            nc.vector.scalar_tensor_tensor(
                xo, acc, b2_sb[:, m: m + 1], x2[:, m, :], OP.add, OP.add)
            nc.sync.dma_start(
                out=t["xoutT"].rearrange("(o p) n -> p o n", p=P)[:, m, :], in_=xo)


_NC_CACHE = {}


def _get_nc():
    if "nc" not in _NC_CACHE:
        _NC_CACHE["nc"] = _build()
    return _NC_CACHE["nc"]


def _prep_shared(inp):
    def bt(a):  # transpose + bf16
        return np.ascontiguousarray(a.T).astype(ml_dtypes.bfloat16)

    w2T = inp["ff_w2"].T  # [DFF, D]
    w2t = np.stack([
        np.ascontiguousarray(
            w2T[:, 128 * m: 128 * m + 128].reshape(32, 128, 128)
            .transpose(1, 0, 2).reshape(128, 4096))
        for m in range(NC)]).astype(ml_dtypes.bfloat16)

    return {
        "w_sa": bt(inp["sa_in_w"]), "b_sa": inp["sa_in_b"],
        "wo_sa": bt(16.0 * inp["sa_out_w"]), "bo_sa": inp["sa_out_b"],
        "w_ca": bt(inp["ca_in_w"]), "b_ca": inp["ca_in_b"],
        "wo_ca": bt(16.0 * inp["ca_out_w"]), "bo_ca": inp["ca_out_b"],
        "w1": bt(inp["ff_w1"]), "b1": inp["ff_b1"],
        "w2t": w2t, "b2": inp["ff_b2"],
        "ln_w": np.ascontiguousarray(
            np.stack([inp["ln1_w"], inp["ln2_w"], inp["ln3_w"]])),
        "ln_b": np.ascontiguousarray(
            np.stack([inp["ln1_b"], inp["ln2_b"], inp["ln3_b"]])),
    }


def _prep_in_maps(inp):
    shared = _prep_shared(inp)
    perms = []
    in_maps = []
    for c in range(8):
        b, r = c // 2, c % 2
        perm = np.r_[512 * r: 512 * r + 512, 512 * (1 - r): 512 * (1 - r) + 512]
        perms.append(perm)
        in_maps.append({
            "xT16": np.ascontiguousarray(inp["x"][b][perm].T).astype(ml_dtypes.bfloat16),
            "xaT16": np.ascontiguousarray(inp["xa"][b].T).astype(ml_dtypes.bfloat16),
            **shared,
        })
    return in_maps, perms


def kernel(**inputs):
    inp = {k: np.asarray(v, dtype=np.float32) for k, v in inputs.items()}
    in_maps, perms = _prep_in_maps(inp)

    res = run_bass_kernel_spmd(_get_nc(), in_maps, core_ids=list(range(8)))

    x = np.empty((B, L, D), np.float32)
    self_w = np.empty((B, L, L), np.float32)
    cross_w = np.empty((B, L, L), np.float32)
    for c in range(8):
        b, r = c // 2, c % 2
        rows = slice(512 * r, 512 * r + 512)
        x[b, rows] = res.results[c]["xoutT"].T
        # b (int) + perm (array) are both advanced indices separated by a
        # slice, so numpy puts the perm dim first: target shape (1024, 512)
        # with semantics self_w[b, l, perm[j]] = selfwT[j, l].
        self_w[b, rows.start: rows.stop, perms[c]] = res.results[c]["selfwT"]
        cross_w[b, rows] = res.results[c]["crosswT"].T
    return (x, self_w, cross_w)
